# revision 12
# baseline (speedup 1.0000x reference)
"""DCTHFClip kernel for 8 Trainium2 NeuronCores — recon-direct edition.

Math: the reference computes
    x_dct   = C @ x          (DCT-II along S, per (batch, feature) column)
    m       = |mean_{b,d} x_dct|          (shape (S,))
    thr     = quantile(m, 0.7); last_index = last k with m[k] > thr
    trunc   = x_dct[:, :L, :]                           (fp32 output)
    recon   = Cl^T @ trunc  with Cl = dct_matrix(L)     (fp16 output)

Design (per core, Bc=8 batches, S=576, D=1024, L resolved on host via
linearity of the batch/feature mean):
  1. HOST butterfly: u = x[:288] + x[575:287:-1], v = x[:288] - ...
     shipped as fp16 (same bytes as x, zero device cost).  Frequency
     parity: trunc[2j] = (Ce @ u)[j], trunc[2j+1] = (Co @ v)[j].
  2. RECON DIRECTLY FROM u/v: with R = Cl^T @ C[:L]  (centro-symmetric:
     R[L-1-p, 575-s] = R[p, s]), A[p,s] = R[p,s], B[p,s] = R[L-1-p,s]:
         g = ((A+B)/2) @ u,   h = ((A-B)/2) @ v
         recon[p] = g + h  (p<FE),   recon[L-1-p] = g - h.
     The final +- runs on HOST from the fp16 g/h outputs, so the device
     never couples the two transforms: every matmul depends only on the
     input DMA.  (Also ~1.4x more accurate than chaining through an
     fp16 spectrum.)
  3. Spectrum pass A is weights-stationary [K=i, M=j(96-chunks), N=d],
     2 PSUM banks per tile.  Recon pass B is transposed [K=i,
     M=d(8x128), N=p(288)] writing g^T/h^T, 1 bank per tile —
     perfect M tiling, no N rounding (host un-transposes).
  4. K = 288 = 128 + 128 + 32.  The 32-row remainders are REPLICATED at
     partition offsets 0/32/64/96; four tail matmuls on disjoint
     row-groups issue back-to-back and execute CONCURRENTLY
     (tile_position row packing), costing ~1 matmul instead of 4.
  5. PE floor ~24.7K moving-cycles/batch at the measured 2.0 GHz
     (P0) clock ~= 99 us/core; evacuation is split ACT/DVE to ~70%
     occupancy each; rings split sync/scalar/gpsimd.
"""

import os
import sys

import numpy as np

_B, _S, _D = 64, 576, 1024
_NCORES = 8
_P = 128
_JT = 96          # stage-A output row chunk
_KM = 128         # main contraction tile
_KR = 32          # contraction remainder (replicated 4x)

_CACHE = {}
LAST_RESULTS = None  # stashed BassKernelResults for test.py profiling


def _ensure_paths():
    for p in ("/root/.axon_site", "/root/.axon_site/_ro/trn_rl_repo",
              "/root/.axon_site/_ro/pypackages", "/opt/trn_rl_repo", "/opt/pypackages"):
        if os.path.isdir(p) and p not in sys.path:
            sys.path.append(p)


def _dct_matrix64(n):
    k = np.arange(n)[:, None].astype(np.float64)
    i = np.arange(n)[None, :].astype(np.float64)
    C = np.cos(np.pi / n * (i + 0.5) * k)
    scale = np.where(k == 0, np.sqrt(1.0 / n), np.sqrt(2.0 / n))
    return C * scale  # (n_freq, n_pos)


def _resolve_L(x):
    """Host-side: trunc length via linearity of the batch/feature mean."""
    S = x.shape[1]
    xbar = x.mean(axis=(0, 2), dtype=np.float64)  # (S,)
    C = _dct_matrix64(S)
    m = np.abs(C @ xbar)
    thr = np.quantile(m, 0.7)
    idx = np.nonzero(m > thr)[0]
    last_index = int(idx[-1]) if idx.size > 0 else -1
    # mirror python slice semantics of x_dct[:, :last_index, :]
    return len(range(S)[:last_index])


def _chunks(n, c):
    out = []
    s = 0
    while s < n:
        out.append((s, min(c, n - s)))
        s += c
    return out


def _tile4(w):
    """Replicate a (32, m) block at partition offsets 0/32/64/96."""
    return np.concatenate([w, w, w, w], axis=0)


def _build_weights(S, L):
    """fp16 weights.  wAm/wAt: spectrum (Ce^T | Co^T) main/tail k-tiles.
    wBm/wBt: recon (Wg | Wh) main/tail.  Tails replicated 4x on
    partitions for concurrent row-group matmuls."""
    H = S // 2
    FE = (L + 1) // 2
    FO = L // 2
    C = _dct_matrix64(S)
    Cl = _dct_matrix64(L)
    f16 = np.float16
    CeT = C[0:2 * FE:2, :H].T          # (H, FE)
    CoT = C[1:2 * FO:2, :H].T          # (H, FO)
    R = Cl.T @ C[0:L, :]               # (L, S)
    A = R[:FE, 0:H]
    Bm = R[L - FE:L, 0:H][::-1]        # B[p,s] = R[L-1-p, s]
    Wg = ((A + Bm) / 2).T              # (H, FE)
    Wh = ((A - Bm) / 2).T
    wAm = np.zeros((2, 2 * _KM, H), f16)
    wAm[0, :, :FE] = CeT[:2 * _KM]
    wAm[1, :, :FO] = CoT[:2 * _KM]
    wAt = np.zeros((2, 4 * _KR, H), f16)
    wAt[0, :, :FE] = _tile4(CeT[2 * _KM:])
    wAt[1, :, :FO] = _tile4(CoT[2 * _KM:])
    wBm = np.zeros((2, 2 * _KM, H), f16)
    wBm[0, :, :FE] = Wg[:2 * _KM]
    wBm[1, :, :FE] = Wh[:2 * _KM]
    wBt = np.zeros((2, 4 * _KR, H), f16)
    wBt[0, :, :FE] = _tile4(Wg[2 * _KM:])
    wBt[1, :, :FE] = _tile4(Wh[2 * _KM:])
    return {"wAm": wAm, "wAt": wAt, "wBm": wBm, "wBt": wBt}


def _build_program(Bc, S, D, L):
    _ensure_paths()
    import concourse.bacc as bacc
    import concourse.mybir as mybir
    import concourse.tile as tile

    f32 = mybir.dt.float32
    f16 = mybir.dt.float16

    H = S // 2                  # 288
    FE = (L + 1) // 2
    FO = L // 2
    jtsE = _chunks(FE, _JT)
    jtsO = _chunks(FO, _JT)
    NDT = D // _P               # 8 recon d-tiles
    NP = H                      # recon moving width (>= FE, 64B aligned)

    nc = bacc.Bacc("TRN2", target_bir_lowering=False, debug=False,
                   num_devices=_NCORES)
    uvm_d = nc.dram_tensor("uvm", [Bc, 2, 2 * _KM, D], f16,
                           kind="ExternalInput")
    uvt_d = nc.dram_tensor("uvt", [Bc, 2, 4 * _KR, D], f16,
                           kind="ExternalInput")
    wAm_d = nc.dram_tensor("wAm", [2, 2 * _KM, H], f16, kind="ExternalInput")
    wAt_d = nc.dram_tensor("wAt", [2, 4 * _KR, H], f16, kind="ExternalInput")
    wBm_d = nc.dram_tensor("wBm", [2, 2 * _KM, H], f16, kind="ExternalInput")
    wBt_d = nc.dram_tensor("wBt", [2, 4 * _KR, H], f16, kind="ExternalInput")
    tr_d = nc.dram_tensor("tr", [Bc, 2, H, D], f16, kind="ExternalOutput")
    gh_d = nc.dram_tensor("gh", [Bc, 2, D, NP], f16, kind="ExternalOutput")

    with tile.TileContext(nc) as tc:
        with (
            tc.tile_pool(name="wpool", bufs=1) as wpool,
            tc.tile_pool(name="uvpool", bufs=3) as uvpool,
            tc.tile_pool(name="ypool", bufs=2) as ypool,
            tc.tile_pool(name="ghpool", bufs=2) as ghpool,
            tc.tile_pool(name="psA", bufs=2, space="PSUM") as psA,
            tc.tile_pool(name="psB", bufs=4, space="PSUM") as psB,
        ):
            wAm_t = wpool.tile([_KM, 2, 2, H], f16)
            wAt_t = wpool.tile([4 * _KR, 2, H], f16)
            wBm_t = wpool.tile([_KM, 2, 2, H], f16)
            wBt_t = wpool.tile([4 * _KR, 2, H], f16)

            def load_weights(uvt0_t):
                # scalar ring, strict consumption order: the A weights and
                # batch-0 tails gate the first bursts; B weights come last
                nc.scalar.dma_start(
                    out=wAm_t,
                    in_=wAm_d[:, :, :].rearrange("two (it p) j -> p two it j",
                                                 p=_KM))
                nc.scalar.dma_start(
                    out=wAt_t,
                    in_=wAt_d[:, :, :].rearrange("two p j -> p two j"))
                for par in range(2):
                    nc.scalar.dma_start(out=uvt0_t[:, par, :],
                                        in_=uvt_d[0, par])
                nc.scalar.dma_start(
                    out=wBm_t,
                    in_=wBm_d[:, :, :].rearrange("two (it p) j -> p two it j",
                                                 p=_KM))
                nc.scalar.dma_start(
                    out=wBt_t,
                    in_=wBt_d[:, :, :].rearrange("two p j -> p two j"))

            def load_uv(b, split):
                uvm_t = uvpool.tile([_KM, 2, 2, D], f16, tag="uvm")
                uvt_t = uvpool.tile([4 * _KR, 2, D], f16, tag="uvt")
                if split:
                    # lead-in: chunked in consumption order; the batch-0
                    # tails ride the scalar ring inside load_weights()
                    for par in range(2):
                        for it in range(2):
                            nc.sync.dma_start(
                                out=uvm_t[:, par, it, :],
                                in_=uvm_d[b, par,
                                          it * _KM:(it + 1) * _KM, :])
                elif b == 1:
                    nc.gpsimd.dma_start(
                        out=uvm_t,
                        in_=uvm_d[b].rearrange("two (it p) d -> p two it d",
                                               p=_KM))
                    nc.gpsimd.dma_start(
                        out=uvt_t,
                        in_=uvt_d[b].rearrange("two p d -> p two d"))
                else:
                    nc.sync.dma_start(
                        out=uvm_t,
                        in_=uvm_d[b].rearrange("two (it p) d -> p two it d",
                                               p=_KM))
                    nc.scalar.dma_start(
                        out=uvt_t,
                        in_=uvt_d[b].rearrange("two p d -> p two d"))
                return uvm_t, uvt_t

            class Batch:
                """Per-batch chain emitters; groups interleave A and B."""

                def __init__(self, b, uvm_t, uvt_t):
                    self.b = b
                    self.uvm = uvm_t
                    self.uvt = uvt_t
                    self.y = ypool.tile([_JT, 2, 3, D], f16, tag="y")
                    self.gh = ghpool.tile([_P, 2, NDT, NP], f16, tag="gh")
                    self.psa = {}       # (par, jt) -> psum tile
                    self.psb = {}       # (gh, dt) -> psum tile
                    self.evacA = 0      # alternator for evac engine
                    self.r = 0          # row-group cycler for tails

                def a_chain(self, par, jt):
                    jl = (jtsE if par == 0 else jtsO)[jt][1]
                    j0 = jt * _JT
                    ps = psA.tile([_JT, D], f32, tag="psA")
                    self.psa[(par, jt)] = (ps, j0, jl)
                    for ng in range(2):
                        n0 = ng * 512
                        for it in range(2):
                            nc.tensor.matmul(
                                ps[0:jl, n0:n0 + 512],
                                wAm_t[:, par, it, j0:j0 + jl],
                                self.uvm[:, par, it, n0:n0 + 512],
                                start=(it == 0), stop=False)

                def a_tail(self, par, jt, ng):
                    ps, j0, jl = self.psa[(par, jt)]
                    r = self.r
                    self.r = (r + 1) % 4
                    p0 = r * _KR
                    n0 = ng * 512
                    nc.tensor.matmul(
                        ps[0:jl, n0:n0 + 512],
                        wAt_t[p0:p0 + _KR, par, j0:j0 + jl],
                        self.uvt[p0:p0 + _KR, par, n0:n0 + 512],
                        start=False, stop=True, tile_position=(p0, 0))

                def a_evac(self, par, jt):
                    ps, j0, jl = self.psa.pop((par, jt))
                    eng = nc.scalar if self.evacA % 2 == 0 else nc.vector
                    self.evacA += 1
                    if eng is nc.scalar:
                        nc.scalar.copy(self.y[0:jl, par, jt, :], ps[0:jl, :])
                    else:
                        nc.vector.tensor_copy(self.y[0:jl, par, jt, :],
                                              ps[0:jl, :])

                def b_chain(self, dt):
                    d0 = dt * _P
                    for gh in range(2):
                        ps = psB.tile([_P, NP], f32, tag="psB")
                        self.psb[(gh, dt)] = ps
                        for it in range(2):
                            nc.tensor.matmul(
                                ps[:, 0:NP],
                                self.uvm[:, gh, it, d0:d0 + _P],
                                wBm_t[:, gh, it, 0:NP],
                                start=(it == 0), stop=False)

                def b_tail(self, dt, gh):
                    ps = self.psb[(gh, dt)]
                    r = self.r
                    self.r = (r + 1) % 4
                    p0 = r * _KR
                    d0 = dt * _P
                    nc.tensor.matmul(
                        ps[:, 0:NP],
                        self.uvt[p0:p0 + _KR, gh, d0:d0 + _P],
                        wBt_t[p0:p0 + _KR, gh, 0:NP],
                        start=False, stop=True, tile_position=(p0, 0))

                def b_evac(self, dt):
                    pa = self.psb.pop((0, dt))
                    pb = self.psb.pop((1, dt))
                    nc.scalar.copy(self.gh[:, 0, dt, :], pa[:, :])
                    nc.vector.tensor_copy(self.gh[:, 1, dt, :], pb[:, :])

                def tr_out(self, par):
                    eng = nc.sync if (self.b + par) % 2 == 0 else nc.scalar
                    eng.dma_start(
                        out=tr_d[self.b, par].rearrange(
                            "(jt p) d -> p jt d", p=_JT),
                        in_=self.y[:, par, :, :])

                def gh_out(self, dts, eng):
                    d0 = dts[0] * _P
                    d1 = (dts[-1] + 1) * _P
                    for ghi in range(2):
                        eng.dma_start(
                            out=gh_d[self.b, ghi, d0:d1, :].rearrange(
                                "(dt p) m -> p dt m", p=_P),
                            in_=self.gh[:, ghi, dts[0]:dts[-1] + 1, :])

            ap = [(0, 0), (0, 1), (0, 2), (1, 0), (1, 1), (1, 2)]
            if len(jtsE) != 3 or len(jtsO) != 3:       # general L fallback
                ap = [(0, j) for j in range(len(jtsE))] + \
                     [(1, j) for j in range(len(jtsO))]
            agroups = [ap[0:2], ap[2:4], ap[4:6]]
            bgroups = [(0, 1), (2, 3), (4, 5), (6, 7)]

            def agroup(bt, gi):
                for par, jt in agroups[gi]:
                    bt.a_chain(par, jt)
                for par, jt in agroups[gi]:
                    for ng in range(2):
                        bt.a_tail(par, jt, ng)
                for par, jt in agroups[gi]:
                    bt.a_evac(par, jt)
                # parity E done after group 1, O after group 2
                if gi == 1:
                    bt.tr_out(0)
                elif gi == 2:
                    bt.tr_out(1)

            def bgroup(bt, gi, gh_eng=None, gh_dts=None):
                dta, dtb = bgroups[gi]
                bt.b_chain(dta)
                bt.b_chain(dtb)
                for dt in (dta, dtb):
                    for ghi in range(2):
                        bt.b_tail(dt, ghi)
                bt.b_evac(dta)
                bt.b_evac(dtb)
                if gh_dts is not None:
                    bt.gh_out(gh_dts, gh_eng)

            uv0 = load_uv(0, True)
            load_weights(uv0[1])
            uvs = {0: uv0}
            prev = None
            for b in range(Bc):
                bt = Batch(b, *uvs.pop(b))
                last = b == Bc - 1
                # rolling schedule: the trailing B-group of batch b-1 slots
                # in after this batch's first A-group so every psB reuse
                # has a full group of PE work between completion and reuse.
                # Prefetch DMAs are emitted mid-batch so they never queue
                # ahead of the current batch's latency-critical data.
                agroup(bt, 0)
                if prev is not None:
                    bgroup(prev, 3, nc.gpsimd, [4, 5, 6, 7])
                bgroup(bt, 0)
                if b == 0:
                    uvs[1] = load_uv(1, False)
                agroup(bt, 1)
                if b + 2 < Bc:
                    uvs[b + 2] = load_uv(b + 2, False)
                bgroup(bt, 1, nc.gpsimd, [0, 1, 2, 3])
                agroup(bt, 2)
                if not last:
                    bgroup(bt, 2)
                    prev = bt
                else:
                    # drain the final batch's outputs as they appear
                    bgroup(bt, 2, nc.sync, [4, 5])
                    bgroup(bt, 3, nc.scalar, [6, 7])

    nc.compile()
    return nc


def _numpy_fallback(x):
    """Reference math on host — only for unexpected shapes/degenerate L."""
    B, S, D = x.shape
    C = _dct_matrix64(S).astype(np.float32)
    x_dct = np.tensordot(C, x, axes=([1], [1])).transpose(1, 0, 2)  # (B,S,D)
    m = np.abs(x_dct.mean(axis=0).mean(axis=1))
    thr = np.quantile(m, 0.7)
    idx = np.nonzero(m > thr)[0]
    last_index = int(idx[-1]) if idx.size > 0 else -1
    trunc = x_dct[:, :last_index, :]
    L = trunc.shape[1]
    Cl = _dct_matrix64(L).astype(np.float32)
    recon = np.tensordot(Cl.T, trunc, axes=([1], [1])).transpose(1, 0, 2)
    return recon.astype(np.float16), np.ascontiguousarray(trunc)


def kernel(x, _trace=False):
    global LAST_RESULTS
    x = np.ascontiguousarray(np.asarray(x), dtype=np.float32)
    if x.shape != (_B, _S, _D):
        return _numpy_fallback(x)

    L = _resolve_L(x)
    FE = (L + 1) // 2
    # device path assumes 3 j-chunks per parity and FE <= 288
    if L < 8 or L >= _S or not (2 * _JT < FE <= 3 * _JT):
        return _numpy_fallback(x)

    Bc = _B // _NCORES
    key = (Bc, _S, _D, L)
    if key not in _CACHE:
        _CACHE[key] = _build_program(Bc, _S, _D, L)
    nc = _CACHE[key]

    _ensure_paths()
    if not _trace:
        os.environ["BASS_NEVER_TRACE"] = "1"
    from concourse.bass_utils import run_bass_kernel_spmd

    H = _S // 2
    FO = L // 2
    W = _build_weights(_S, L)
    xf = x[:, :H, :]
    xb = x[:, _S - 1:H - 1:-1, :]
    u = (xf + xb).astype(np.float16)
    v = (xf - xb).astype(np.float16)
    uvm = np.empty((_B, 2, 2 * _KM, _D), dtype=np.float16)
    uvm[:, 0] = u[:, :2 * _KM]
    uvm[:, 1] = v[:, :2 * _KM]
    uvt = np.empty((_B, 2, 4 * _KR, _D), dtype=np.float16)
    uvt[:, 0] = np.concatenate([u[:, 2 * _KM:]] * 4, axis=1)
    uvt[:, 1] = np.concatenate([v[:, 2 * _KM:]] * 4, axis=1)
    in_maps = []
    for i in range(_NCORES):
        m = {"uvm": uvm[i * Bc:(i + 1) * Bc], "uvt": uvt[i * Bc:(i + 1) * Bc]}
        m.update(W)
        in_maps.append(m)
    res = run_bass_kernel_spmd(nc, in_maps, list(range(_NCORES)), trace=_trace)
    LAST_RESULTS = res

    trunc = np.empty((_B, L, _D), dtype=np.float32)
    recon = np.empty((_B, L, _D), dtype=np.float16)
    for i in range(_NCORES):
        sl = slice(i * Bc, (i + 1) * Bc)
        tr = res.results[i]["tr"]          # [Bc, 2, H, D] f16
        trunc[sl, 0::2] = tr[:, 0, :FE]
        trunc[sl, 1::2] = tr[:, 1, :FO]
        gh = res.results[i]["gh"]          # [Bc, 2, D, NP] f16
        g = gh[:, 0, :, :FE].transpose(0, 2, 1)
        h = gh[:, 1, :, :FE].transpose(0, 2, 1)
        recon[sl, :FE] = g + h
        recon[sl, FE:] = (g - h)[:, L - 1 - FE::-1]
    return recon, trunc


# revision 20
# speedup vs baseline: 1.0494x; 1.0494x over previous
"""DCTHFClip kernel for 8 Trainium2 NeuronCores — recon-direct edition.

Math: the reference computes
    x_dct   = C @ x          (DCT-II along S, per (batch, feature) column)
    m       = |mean_{b,d} x_dct|          (shape (S,))
    thr     = quantile(m, 0.7); last_index = last k with m[k] > thr
    trunc   = x_dct[:, :L, :]                           (fp32 output)
    recon   = Cl^T @ trunc  with Cl = dct_matrix(L)     (fp16 output)

Design (per core, Bc=8 batches, S=576, D=1024, L resolved on host via
linearity of the batch/feature mean):
  1. HOST butterfly: u = x[:288] + x[575:287:-1], v = x[:288] - ...
     shipped as fp16 (same bytes as x, zero device cost).  Frequency
     parity: trunc[2j] = (Ce @ u)[j], trunc[2j+1] = (Co @ v)[j].
  2. RECON DIRECTLY FROM u/v: with R = Cl^T @ C[:L]  (centro-symmetric:
     R[L-1-p, 575-s] = R[p, s]), A[p,s] = R[p,s], B[p,s] = R[L-1-p,s]:
         g = ((A+B)/2) @ u,   h = ((A-B)/2) @ v
         recon[p] = g + h  (p<FE),   recon[L-1-p] = g - h.
     The final +- runs on HOST from the fp16 g/h outputs, so the device
     never couples the two transforms: every matmul depends only on the
     input DMA.  (Also ~1.4x more accurate than chaining through an
     fp16 spectrum.)
  3. Spectrum pass A is weights-stationary [K=i, M=j(96-chunks), N=d],
     2 PSUM banks per tile.  Recon pass B is transposed [K=i,
     M=d(8x128), N=p(288)] writing g^T/h^T, 1 bank per tile —
     perfect M tiling, no N rounding (host un-transposes).
  4. K = 288 = 128 + 128 + 32.  The 32-row remainders are REPLICATED at
     partition offsets 0/32/64/96; four tail matmuls on disjoint
     row-groups issue back-to-back and execute CONCURRENTLY
     (tile_position row packing), costing ~1 matmul instead of 4.
  5. PE floor ~24.7K moving-cycles/batch at the measured 2.0 GHz
     (P0) clock ~= 99 us/core; evacuation is split ACT/DVE to ~70%
     occupancy each; rings split sync/scalar/gpsimd.
"""

import os
import sys

import numpy as np

_B, _S, _D = 64, 576, 1024
_NCORES = 8
_P = 128
_JT = 96          # stage-A output row chunk
_KM = 128         # main contraction tile
_KR = 32          # contraction remainder (replicated 4x)

_CACHE = {}
LAST_RESULTS = None  # stashed BassKernelResults for test.py profiling


def _ensure_paths():
    for p in ("/root/.axon_site", "/root/.axon_site/_ro/trn_rl_repo",
              "/root/.axon_site/_ro/pypackages", "/opt/trn_rl_repo", "/opt/pypackages"):
        if os.path.isdir(p) and p not in sys.path:
            sys.path.append(p)


def _dct_matrix64(n):
    k = np.arange(n)[:, None].astype(np.float64)
    i = np.arange(n)[None, :].astype(np.float64)
    C = np.cos(np.pi / n * (i + 0.5) * k)
    scale = np.where(k == 0, np.sqrt(1.0 / n), np.sqrt(2.0 / n))
    return C * scale  # (n_freq, n_pos)


def _resolve_L(x):
    """Host-side: trunc length via linearity of the batch/feature mean."""
    S = x.shape[1]
    xbar = x.mean(axis=(0, 2), dtype=np.float64)  # (S,)
    C = _dct_matrix64(S)
    m = np.abs(C @ xbar)
    thr = np.quantile(m, 0.7)
    idx = np.nonzero(m > thr)[0]
    last_index = int(idx[-1]) if idx.size > 0 else -1
    # mirror python slice semantics of x_dct[:, :last_index, :]
    return len(range(S)[:last_index])


def _chunks(n, c):
    out = []
    s = 0
    while s < n:
        out.append((s, min(c, n - s)))
        s += c
    return out


def _tile4(w):
    """Replicate a (32, m) block at partition offsets 0/32/64/96."""
    return np.concatenate([w, w, w, w], axis=0)


def _pack_kt(w):
    """(288, m) -> (384, m): two 128-row main k-tiles + the 32-row tail
    replicated at partition offsets 0/32/64/96 of the third tile."""
    return np.concatenate([w[:2 * _KM], _tile4(w[2 * _KM:])], axis=0)


def _build_weights(S, L):
    """fp16 weights, k-tile packed.  wA[par]: spectrum Ce^T | Co^T.
    wB[par]: recon Wg | Wh (g from u, h from v)."""
    H = S // 2
    FE = (L + 1) // 2
    FO = L // 2
    C = _dct_matrix64(S)
    Cl = _dct_matrix64(L)
    f16 = np.float16
    CeT = C[0:2 * FE:2, :H].T          # (H, FE)
    CoT = C[1:2 * FO:2, :H].T          # (H, FO)
    R = Cl.T @ C[0:L, :]               # (L, S)
    A = R[:FE, 0:H]
    Bm = R[L - FE:L, 0:H][::-1]        # B[p,s] = R[L-1-p, s]
    Wg = ((A + Bm) / 2).T              # (H, FE)
    Wh = ((A - Bm) / 2).T
    wA = np.zeros((2, 3 * _KM, H), f16)
    wA[0, :, :FE] = _pack_kt(CeT)
    wA[1, :, :FO] = _pack_kt(CoT)
    wB = np.zeros((2, 3 * _KM, H), f16)
    wB[0, :, :FE] = _pack_kt(Wg)
    wB[1, :, :FE] = _pack_kt(Wh)
    return {"wA": wA, "wB": wB}


def _build_program(Bc, S, D, L):
    _ensure_paths()
    import concourse.bacc as bacc
    import concourse.mybir as mybir
    import concourse.tile as tile

    f32 = mybir.dt.float32
    f16 = mybir.dt.float16

    H = S // 2                  # 288
    FE = (L + 1) // 2
    FO = L // 2
    jtsE = _chunks(FE, _JT)
    jtsO = _chunks(FO, _JT)
    NDT = D // _P               # 8 recon d-tiles
    NP = H                      # recon moving width (>= FE, 64B aligned)

    nc = bacc.Bacc("TRN2", target_bir_lowering=False, debug=False,
                   num_devices=_NCORES)
    uv_d = nc.dram_tensor("uv", [Bc, 2, 3 * _KM, D], f16,
                          kind="ExternalInput")
    wA_d = nc.dram_tensor("wA", [2, 3 * _KM, H], f16, kind="ExternalInput")
    wB_d = nc.dram_tensor("wB", [2, 3 * _KM, H], f16, kind="ExternalInput")
    tr_d = nc.dram_tensor("tr", [Bc, 2, H, D], f16, kind="ExternalOutput")
    gh_d = nc.dram_tensor("gh", [Bc, D, 2 * NP], f16, kind="ExternalOutput")

    with tile.TileContext(nc) as tc:
        with (
            tc.tile_pool(name="wpool", bufs=1) as wpool,
            tc.tile_pool(name="uvpool", bufs=3) as uvpool,
            tc.tile_pool(name="ypool", bufs=2) as ypool,
            tc.tile_pool(name="ghpool", bufs=2) as ghpool,
            tc.tile_pool(name="psA", bufs=2, space="PSUM") as psA,
            tc.tile_pool(name="psB", bufs=4, space="PSUM") as psB,
        ):
            wA_t = wpool.tile([_KM, 2, 3, H], f16)
            wB_t = wpool.tile([_KM, 2, 3, H], f16)

            def load_weights():
                # scalar ring: wA gates the very first chains
                for t, d_ in ((wA_t, wA_d), (wB_t, wB_d)):
                    nc.scalar.dma_start(
                        out=t,
                        in_=d_[:, :, :].rearrange("two (it p) j -> p two it j",
                                                  p=_KM))

            def load_uv(b, split):
                uv_t = uvpool.tile([_KM, 2, 3, D], f16, tag="uv")
                if split:
                    # lead-in: par 0 chunked on sync (first chains need
                    # it0 only), par 1 rides the idle SWDGE ring
                    nc.sync.dma_start(out=uv_t[:, 0, 0, :],
                                      in_=uv_d[b, 0, 0:_KM, :])
                    nc.sync.dma_start(
                        out=uv_t[:, 0, 1:3, :],
                        in_=uv_d[b, 0, _KM:3 * _KM, :].rearrange(
                            "(it p) d -> p it d", p=_KM))
                    nc.gpsimd.dma_start(
                        out=uv_t[:, 1, :, :],
                        in_=uv_d[b, 1].rearrange("(it p) d -> p it d",
                                                 p=_KM))
                else:
                    nc.sync.dma_start(
                        out=uv_t,
                        in_=uv_d[b].rearrange("two (it p) d -> p two it d",
                                              p=_KM))
                return uv_t

            class Batch:
                """Per-batch chain emitters; groups interleave A and B."""

                def __init__(self, b, uv_t):
                    self.b = b
                    self.uv = uv_t
                    self.y = ypool.tile([_JT, 2, 3, D], f16, tag="y")
                    self.gh = ghpool.tile([_P, NDT, 2 * NP], f16, tag="gh")
                    self.psa = {}       # (par, jt) -> psum tile
                    self.psb = {}       # (gh, dt) -> psum tile
                    self.evacA = 0      # alternator for evac engine
                    self.r = 0          # row-group cycler for tails

                def a_chain(self, par, jt):
                    jl = (jtsE if par == 0 else jtsO)[jt][1]
                    j0 = jt * _JT
                    ps = psA.tile([_JT, D], f32, tag="psA")
                    self.psa[(par, jt)] = (ps, j0, jl)
                    for ng in range(2):
                        n0 = ng * 512
                        for it in range(2):
                            nc.tensor.matmul(
                                ps[0:jl, n0:n0 + 512],
                                wA_t[:, par, it, j0:j0 + jl],
                                self.uv[:, par, it, n0:n0 + 512],
                                start=(it == 0), stop=False)

                def a_tail(self, par, jt, ng):
                    ps, j0, jl = self.psa[(par, jt)]
                    r = self.r
                    self.r = (r + 1) % 4
                    p0 = r * _KR
                    n0 = ng * 512
                    nc.tensor.matmul(
                        ps[0:jl, n0:n0 + 512],
                        wA_t[p0:p0 + _KR, par, 2, j0:j0 + jl],
                        self.uv[p0:p0 + _KR, par, 2, n0:n0 + 512],
                        start=False, stop=True, tile_position=(p0, 0))

                def a_evac(self, par, jt):
                    ps, j0, jl = self.psa.pop((par, jt))
                    eng = nc.scalar if self.evacA % 2 == 0 else nc.vector
                    self.evacA += 1
                    if eng is nc.scalar:
                        nc.scalar.copy(self.y[0:jl, par, jt, :], ps[0:jl, :])
                    else:
                        nc.vector.tensor_copy(self.y[0:jl, par, jt, :],
                                              ps[0:jl, :])

                def b_chain(self, dt):
                    d0 = dt * _P
                    for ghi in range(2):
                        ps = psB.tile([_P, NP], f32, tag="psB")
                        self.psb[(ghi, dt)] = ps
                        for it in range(2):
                            nc.tensor.matmul(
                                ps[:, 0:NP],
                                self.uv[:, ghi, it, d0:d0 + _P],
                                wB_t[:, ghi, it, 0:NP],
                                start=(it == 0), stop=False)

                def b_tail(self, dt, ghi):
                    ps = self.psb[(ghi, dt)]
                    r = self.r
                    self.r = (r + 1) % 4
                    p0 = r * _KR
                    d0 = dt * _P
                    nc.tensor.matmul(
                        ps[:, 0:NP],
                        self.uv[p0:p0 + _KR, ghi, 2, d0:d0 + _P],
                        wB_t[p0:p0 + _KR, ghi, 2, 0:NP],
                        start=False, stop=True, tile_position=(p0, 0))

                def b_evac(self, dt):
                    pa = self.psb.pop((0, dt))
                    pb = self.psb.pop((1, dt))
                    nc.scalar.copy(self.gh[:, dt, 0:NP], pa[:, :])
                    nc.vector.tensor_copy(self.gh[:, dt, NP:2 * NP],
                                          pb[:, :])

                def tr_out(self, par):
                    eng = nc.scalar if par == 0 else nc.gpsimd
                    eng.dma_start(
                        out=tr_d[self.b, par].rearrange(
                            "(jt p) d -> p jt d", p=_JT),
                        in_=self.y[:, par, :, :])

                def gh_out(self, dts, eng):
                    d0 = dts[0] * _P
                    d1 = (dts[-1] + 1) * _P
                    eng.dma_start(
                        out=gh_d[self.b, d0:d1, :].rearrange(
                            "(dt p) m -> p dt m", p=_P),
                        in_=self.gh[:, dts[0]:dts[-1] + 1, :])

            ap = [(0, 0), (0, 1), (0, 2), (1, 0), (1, 1), (1, 2)]
            if len(jtsE) != 3 or len(jtsO) != 3:       # general L fallback
                ap = [(0, j) for j in range(len(jtsE))] + \
                     [(1, j) for j in range(len(jtsO))]
            agroups = [ap[0:2], ap[2:4], ap[4:6]]
            bgroups = [(0, 1), (2, 3), (4, 5), (6, 7)]

            def agroup(bt, gi):
                for par, jt in agroups[gi]:
                    bt.a_chain(par, jt)
                for par, jt in agroups[gi]:
                    for ng in range(2):
                        bt.a_tail(par, jt, ng)
                for par, jt in agroups[gi]:
                    bt.a_evac(par, jt)
                # parity E done after group 1, O after group 2
                if gi == 1:
                    bt.tr_out(0)
                elif gi == 2:
                    bt.tr_out(1)

            def bgroup(bt, gi, gh_eng=None, gh_dts=None):
                dta, dtb = bgroups[gi]
                bt.b_chain(dta)
                bt.b_chain(dtb)
                for dt in (dta, dtb):
                    for ghi in range(2):
                        bt.b_tail(dt, ghi)
                bt.b_evac(dta)
                bt.b_evac(dtb)
                if gh_dts is not None:
                    bt.gh_out(gh_dts, gh_eng)

            uv0 = load_uv(0, True)
            load_weights()
            uvs = {0: uv0}
            prev = None
            for b in range(Bc):
                bt = Batch(b, uvs.pop(b))
                last = b == Bc - 1
                # rolling schedule: the trailing B-group of batch b-1 slots
                # in after this batch's first A-group so every psB reuse
                # has a full group of PE work between completion and reuse.
                # Prefetch DMAs are emitted mid-batch so they never queue
                # ahead of the current batch's latency-critical data.
                agroup(bt, 0)
                if prev is not None:
                    bgroup(prev, 3, nc.gpsimd, [4, 5, 6, 7])
                bgroup(bt, 0)
                if b == 0:
                    uvs[1] = load_uv(1, False)
                agroup(bt, 1)
                if b + 2 < Bc:
                    uvs[b + 2] = load_uv(b + 2, False)
                bgroup(bt, 1, nc.gpsimd, [0, 1, 2, 3])
                agroup(bt, 2)
                if not last:
                    bgroup(bt, 2)
                    prev = bt
                else:
                    # drain the final batch's outputs as they appear
                    bgroup(bt, 2, nc.sync, [4, 5])
                    bgroup(bt, 3, nc.scalar, [6, 7])

    nc.compile()
    return nc


def _numpy_fallback(x):
    """Reference math on host — only for unexpected shapes/degenerate L."""
    B, S, D = x.shape
    C = _dct_matrix64(S).astype(np.float32)
    x_dct = np.tensordot(C, x, axes=([1], [1])).transpose(1, 0, 2)  # (B,S,D)
    m = np.abs(x_dct.mean(axis=0).mean(axis=1))
    thr = np.quantile(m, 0.7)
    idx = np.nonzero(m > thr)[0]
    last_index = int(idx[-1]) if idx.size > 0 else -1
    trunc = x_dct[:, :last_index, :]
    L = trunc.shape[1]
    Cl = _dct_matrix64(L).astype(np.float32)
    recon = np.tensordot(Cl.T, trunc, axes=([1], [1])).transpose(1, 0, 2)
    return recon.astype(np.float16), np.ascontiguousarray(trunc)


def kernel(x, _trace=False):
    global LAST_RESULTS
    x = np.ascontiguousarray(np.asarray(x), dtype=np.float32)
    if x.shape != (_B, _S, _D):
        return _numpy_fallback(x)

    L = _resolve_L(x)
    FE = (L + 1) // 2
    # device path assumes 3 j-chunks per parity and FE <= 288
    if L < 8 or L >= _S or not (2 * _JT < FE <= 3 * _JT):
        return _numpy_fallback(x)

    Bc = _B // _NCORES
    key = (Bc, _S, _D, L)
    if key not in _CACHE:
        _CACHE[key] = _build_program(Bc, _S, _D, L)
    nc = _CACHE[key]

    _ensure_paths()
    if not _trace:
        os.environ["BASS_NEVER_TRACE"] = "1"
    from concourse.bass_utils import run_bass_kernel_spmd

    H = _S // 2
    FO = L // 2
    W = _build_weights(_S, L)
    xf = x[:, :H, :]
    xb = x[:, _S - 1:H - 1:-1, :]
    u = (xf + xb).astype(np.float16)
    v = (xf - xb).astype(np.float16)
    uv = np.empty((_B, 2, 3 * _KM, _D), dtype=np.float16)
    for pi, arr in ((0, u), (1, v)):
        uv[:, pi, :2 * _KM] = arr[:, :2 * _KM]
        for r in range(4):
            uv[:, pi, 2 * _KM + r * _KR:2 * _KM + (r + 1) * _KR] = \
                arr[:, 2 * _KM:]
    in_maps = []
    for i in range(_NCORES):
        m = {"uv": uv[i * Bc:(i + 1) * Bc]}
        m.update(W)
        in_maps.append(m)
    res = run_bass_kernel_spmd(nc, in_maps, list(range(_NCORES)), trace=_trace)
    LAST_RESULTS = res

    trunc = np.empty((_B, L, _D), dtype=np.float32)
    recon = np.empty((_B, L, _D), dtype=np.float16)
    for i in range(_NCORES):
        sl = slice(i * Bc, (i + 1) * Bc)
        tr = res.results[i]["tr"]          # [Bc, 2, H, D] f16
        trunc[sl, 0::2] = tr[:, 0, :FE]
        trunc[sl, 1::2] = tr[:, 1, :FO]
        gh = res.results[i]["gh"]          # [Bc, D, 2*NP] f16
        g = gh[:, :, :FE].transpose(0, 2, 1)
        h = gh[:, :, H:H + FE].transpose(0, 2, 1)
        recon[sl, :FE] = g + h
        recon[sl, FE:] = (g - h)[:, L - 1 - FE::-1]
    return recon, trunc


# revision 22
# speedup vs baseline: 1.0629x; 1.0129x over previous
"""DCTHFClip kernel for 8 Trainium2 NeuronCores — recon-direct edition.

Math: the reference computes
    x_dct   = C @ x          (DCT-II along S, per (batch, feature) column)
    m       = |mean_{b,d} x_dct|          (shape (S,))
    thr     = quantile(m, 0.7); last_index = last k with m[k] > thr
    trunc   = x_dct[:, :L, :]                           (fp32 output)
    recon   = Cl^T @ trunc  with Cl = dct_matrix(L)     (fp16 output)

Design (per core, Bc=8 batches, S=576, D=1024, L resolved on host via
linearity of the batch/feature mean):
  1. HOST butterfly: u = x[:288] + x[575:287:-1], v = x[:288] - ...
     shipped as fp16 (same bytes as x, zero device cost).  Frequency
     parity: trunc[2j] = (Ce @ u)[j], trunc[2j+1] = (Co @ v)[j].
  2. RECON DIRECTLY FROM u/v: with R = Cl^T @ C[:L]  (centro-symmetric:
     R[L-1-p, 575-s] = R[p, s]), A[p,s] = R[p,s], B[p,s] = R[L-1-p,s]:
         g = ((A+B)/2) @ u,   h = ((A-B)/2) @ v
         recon[p] = g + h  (p<FE),   recon[L-1-p] = g - h.
     The final +- runs on HOST from the fp16 g/h outputs, so the device
     never couples the two transforms: every matmul depends only on the
     input DMA.  (Also ~1.4x more accurate than chaining through an
     fp16 spectrum.)
  3. Spectrum pass A is weights-stationary [K=i, M=j(96-chunks), N=d],
     2 PSUM banks per tile.  Recon pass B is transposed [K=i,
     M=d(8x128), N=p(288)] writing g^T/h^T, 1 bank per tile —
     perfect M tiling, no N rounding (host un-transposes).
  4. K = 288 = 128 + 128 + 32.  The 32-row remainders are REPLICATED at
     partition offsets 0/32/64/96; four tail matmuls on disjoint
     row-groups issue back-to-back and execute CONCURRENTLY
     (tile_position row packing), costing ~1 matmul instead of 4.
  5. PE floor ~24.7K moving-cycles/batch at the measured 2.0 GHz
     (P0) clock ~= 99 us/core; evacuation is split ACT/DVE to ~70%
     occupancy each; rings split sync/scalar/gpsimd.
"""

import os
import sys

import numpy as np

_B, _S, _D = 64, 576, 1024
_NCORES = 8
_P = 128
_JT = 96          # stage-A output row chunk
_KM = 128         # main contraction tile
_KR = 32          # contraction remainder (replicated 4x)

_CACHE = {}
LAST_RESULTS = None  # stashed BassKernelResults for test.py profiling


def _ensure_paths():
    for p in ("/root/.axon_site", "/root/.axon_site/_ro/trn_rl_repo",
              "/root/.axon_site/_ro/pypackages", "/opt/trn_rl_repo", "/opt/pypackages"):
        if os.path.isdir(p) and p not in sys.path:
            sys.path.append(p)


def _dct_matrix64(n):
    k = np.arange(n)[:, None].astype(np.float64)
    i = np.arange(n)[None, :].astype(np.float64)
    C = np.cos(np.pi / n * (i + 0.5) * k)
    scale = np.where(k == 0, np.sqrt(1.0 / n), np.sqrt(2.0 / n))
    return C * scale  # (n_freq, n_pos)


def _resolve_L(x):
    """Host-side: trunc length via linearity of the batch/feature mean."""
    S = x.shape[1]
    xbar = x.mean(axis=(0, 2), dtype=np.float64)  # (S,)
    C = _dct_matrix64(S)
    m = np.abs(C @ xbar)
    thr = np.quantile(m, 0.7)
    idx = np.nonzero(m > thr)[0]
    last_index = int(idx[-1]) if idx.size > 0 else -1
    # mirror python slice semantics of x_dct[:, :last_index, :]
    return len(range(S)[:last_index])


def _chunks(n, c):
    out = []
    s = 0
    while s < n:
        out.append((s, min(c, n - s)))
        s += c
    return out


def _tile4(w):
    """Replicate a (32, m) block at partition offsets 0/32/64/96."""
    return np.concatenate([w, w, w, w], axis=0)


def _pack_kt(w):
    """(288, m) -> (384, m): two 128-row main k-tiles + the 32-row tail
    replicated at partition offsets 0/32/64/96 of the third tile."""
    return np.concatenate([w[:2 * _KM], _tile4(w[2 * _KM:])], axis=0)


def _build_weights(S, L):
    """fp16 weights, k-tile packed.  wA[par]: spectrum Ce^T | Co^T.
    wB[par]: recon Wg | Wh (g from u, h from v)."""
    H = S // 2
    FE = (L + 1) // 2
    FO = L // 2
    C = _dct_matrix64(S)
    Cl = _dct_matrix64(L)
    f16 = np.float16
    CeT = C[0:2 * FE:2, :H].T          # (H, FE)
    CoT = C[1:2 * FO:2, :H].T          # (H, FO)
    R = Cl.T @ C[0:L, :]               # (L, S)
    A = R[:FE, 0:H]
    Bm = R[L - FE:L, 0:H][::-1]        # B[p,s] = R[L-1-p, s]
    Wg = ((A + Bm) / 2).T              # (H, FE)
    Wh = ((A - Bm) / 2).T
    wA = np.zeros((2, 3 * _KM, H), f16)
    wA[0, :, :FE] = _pack_kt(CeT)
    wA[1, :, :FO] = _pack_kt(CoT)
    wB = np.zeros((2, 3 * _KM, H), f16)
    wB[0, :, :FE] = _pack_kt(Wg)
    wB[1, :, :FE] = _pack_kt(Wh)
    return {"wA": wA, "wB": wB}


def _build_program(Bc, S, D, L):
    _ensure_paths()
    import concourse.bacc as bacc
    import concourse.mybir as mybir
    import concourse.tile as tile

    f32 = mybir.dt.float32
    f16 = mybir.dt.float16

    H = S // 2                  # 288
    FE = (L + 1) // 2
    FO = L // 2
    jtsE = _chunks(FE, _JT)
    jtsO = _chunks(FO, _JT)
    NDT = D // _P               # 8 recon d-tiles
    NP = H                      # recon moving width (>= FE, 64B aligned)

    nc = bacc.Bacc("TRN2", target_bir_lowering=False, debug=False,
                   num_devices=_NCORES)
    uv_d = nc.dram_tensor("uv", [Bc, 2, 3 * _KM, D], f16,
                          kind="ExternalInput")
    wA_d = nc.dram_tensor("wA", [2, 3 * _KM, H], f16, kind="ExternalInput")
    wB_d = nc.dram_tensor("wB", [2, 3 * _KM, H], f16, kind="ExternalInput")
    tr_d = nc.dram_tensor("tr", [Bc, 2, H, D], f16, kind="ExternalOutput")
    gh_d = nc.dram_tensor("gh", [Bc, D, 2 * NP], f16, kind="ExternalOutput")

    with tile.TileContext(nc) as tc:
        with (
            tc.tile_pool(name="wpool", bufs=1) as wpool,
            tc.tile_pool(name="uvpool", bufs=3) as uvpool,
            tc.tile_pool(name="ypool", bufs=2) as ypool,
            tc.tile_pool(name="ghpool", bufs=2) as ghpool,
            tc.tile_pool(name="psA", bufs=2, space="PSUM") as psA,
            tc.tile_pool(name="psB", bufs=4, space="PSUM") as psB,
        ):
            wA_t = wpool.tile([_KM, 2, 3, H], f16)
            wB_t = wpool.tile([_KM, 2, 3, H], f16)

            def load_weights():
                # scalar ring: wA gates the very first chains
                for t, d_ in ((wA_t, wA_d), (wB_t, wB_d)):
                    nc.scalar.dma_start(
                        out=t,
                        in_=d_[:, :, :].rearrange("two (it p) j -> p two it j",
                                                  p=_KM))

            def load_uv(b, split):
                uv_t = uvpool.tile([_KM, 2, 3, D], f16, tag="uv")
                if split:
                    # lead-in: par 0 chunked per k-tile on sync (first
                    # chains need it0 only), par 1 on the idle SWDGE ring
                    for it in range(3):
                        nc.sync.dma_start(
                            out=uv_t[:, 0, it, :],
                            in_=uv_d[b, 0, it * _KM:(it + 1) * _KM, :])
                    nc.gpsimd.dma_start(
                        out=uv_t[:, 1, :, :],
                        in_=uv_d[b, 1].rearrange("(it p) d -> p it d",
                                                 p=_KM))
                else:
                    nc.sync.dma_start(
                        out=uv_t,
                        in_=uv_d[b].rearrange("two (it p) d -> p two it d",
                                              p=_KM))
                return uv_t

            class Batch:
                """Per-batch chain emitters; groups interleave A and B."""

                def __init__(self, b, uv_t):
                    self.b = b
                    self.uv = uv_t
                    self.y = ypool.tile([_JT, 2, 3, D], f16, tag="y")
                    self.gh = ghpool.tile([_P, NDT, 2 * NP], f16, tag="gh")
                    self.psa = {}       # (par, jt) -> psum tile
                    self.psb = {}       # (gh, dt) -> psum tile
                    self.evacA = 0      # alternator for evac engine
                    self.r = 0          # row-group cycler for tails

                def a_chain(self, par, jt):
                    jl = (jtsE if par == 0 else jtsO)[jt][1]
                    j0 = jt * _JT
                    ps = psA.tile([_JT, D], f32, tag="psA")
                    self.psa[(par, jt)] = (ps, j0, jl)
                    # it-major: both ng chains consume k-tile `it` before
                    # moving on, so a chunked uv load feeds 2 MMs at once
                    for it in range(2):
                        for ng in range(2):
                            n0 = ng * 512
                            nc.tensor.matmul(
                                ps[0:jl, n0:n0 + 512],
                                wA_t[:, par, it, j0:j0 + jl],
                                self.uv[:, par, it, n0:n0 + 512],
                                start=(it == 0), stop=False)

                def a_tail(self, par, jt, ng):
                    ps, j0, jl = self.psa[(par, jt)]
                    r = self.r
                    self.r = (r + 1) % 4
                    p0 = r * _KR
                    n0 = ng * 512
                    nc.tensor.matmul(
                        ps[0:jl, n0:n0 + 512],
                        wA_t[p0:p0 + _KR, par, 2, j0:j0 + jl],
                        self.uv[p0:p0 + _KR, par, 2, n0:n0 + 512],
                        start=False, stop=True, tile_position=(p0, 0))

                def a_evac(self, par, jt):
                    ps, j0, jl = self.psa.pop((par, jt))
                    eng = nc.scalar if self.evacA % 2 == 0 else nc.vector
                    self.evacA += 1
                    if eng is nc.scalar:
                        nc.scalar.copy(self.y[0:jl, par, jt, :], ps[0:jl, :])
                    else:
                        nc.vector.tensor_copy(self.y[0:jl, par, jt, :],
                                              ps[0:jl, :])

                def b_chain(self, dt):
                    d0 = dt * _P
                    for ghi in range(2):
                        ps = psB.tile([_P, NP], f32, tag="psB")
                        self.psb[(ghi, dt)] = ps
                        for it in range(2):
                            nc.tensor.matmul(
                                ps[:, 0:NP],
                                self.uv[:, ghi, it, d0:d0 + _P],
                                wB_t[:, ghi, it, 0:NP],
                                start=(it == 0), stop=False)

                def b_tail(self, dt, ghi):
                    ps = self.psb[(ghi, dt)]
                    r = self.r
                    self.r = (r + 1) % 4
                    p0 = r * _KR
                    d0 = dt * _P
                    nc.tensor.matmul(
                        ps[:, 0:NP],
                        self.uv[p0:p0 + _KR, ghi, 2, d0:d0 + _P],
                        wB_t[p0:p0 + _KR, ghi, 2, 0:NP],
                        start=False, stop=True, tile_position=(p0, 0))

                def b_evac(self, dt):
                    pa = self.psb.pop((0, dt))
                    pb = self.psb.pop((1, dt))
                    nc.scalar.copy(self.gh[:, dt, 0:NP], pa[:, :])
                    nc.vector.tensor_copy(self.gh[:, dt, NP:2 * NP],
                                          pb[:, :])

                def tr_out(self, par):
                    eng = nc.scalar if par == 0 else nc.gpsimd
                    eng.dma_start(
                        out=tr_d[self.b, par].rearrange(
                            "(jt p) d -> p jt d", p=_JT),
                        in_=self.y[:, par, :, :])

                def gh_out(self, dts, eng):
                    d0 = dts[0] * _P
                    d1 = (dts[-1] + 1) * _P
                    eng.dma_start(
                        out=gh_d[self.b, d0:d1, :].rearrange(
                            "(dt p) m -> p dt m", p=_P),
                        in_=self.gh[:, dts[0]:dts[-1] + 1, :])

            ap = [(0, 0), (0, 1), (0, 2), (1, 0), (1, 1), (1, 2)]
            if len(jtsE) != 3 or len(jtsO) != 3:       # general L fallback
                ap = [(0, j) for j in range(len(jtsE))] + \
                     [(1, j) for j in range(len(jtsO))]
            agroups = [ap[0:2], ap[2:4], ap[4:6]]
            bgroups = [(0, 1), (2, 3), (4, 5), (6, 7)]

            def agroup(bt, gi):
                for par, jt in agroups[gi]:
                    bt.a_chain(par, jt)
                for par, jt in agroups[gi]:
                    for ng in range(2):
                        bt.a_tail(par, jt, ng)
                for par, jt in agroups[gi]:
                    bt.a_evac(par, jt)
                # parity E done after group 1, O after group 2
                if gi == 1:
                    bt.tr_out(0)
                elif gi == 2:
                    bt.tr_out(1)

            def bgroup(bt, gi, gh_eng=None, gh_dts=None):
                dta, dtb = bgroups[gi]
                bt.b_chain(dta)
                bt.b_chain(dtb)
                for dt in (dta, dtb):
                    for ghi in range(2):
                        bt.b_tail(dt, ghi)
                bt.b_evac(dta)
                bt.b_evac(dtb)
                if gh_dts is not None:
                    bt.gh_out(gh_dts, gh_eng)

            uv0 = load_uv(0, True)
            load_weights()
            uvs = {0: uv0}
            prev = None
            for b in range(Bc):
                bt = Batch(b, uvs.pop(b))
                last = b == Bc - 1
                # rolling schedule: the trailing B-group of batch b-1 slots
                # in after this batch's first A-group so every psB reuse
                # has a full group of PE work between completion and reuse.
                # Prefetch DMAs are emitted mid-batch so they never queue
                # ahead of the current batch's latency-critical data.
                agroup(bt, 0)
                if prev is not None:
                    bgroup(prev, 3, nc.gpsimd, [4, 5, 6, 7])
                bgroup(bt, 0)
                if b == 0:
                    uvs[1] = load_uv(1, False)
                agroup(bt, 1)
                if b + 2 < Bc:
                    uvs[b + 2] = load_uv(b + 2, False)
                bgroup(bt, 1, nc.gpsimd, [0, 1, 2, 3])
                agroup(bt, 2)
                if not last:
                    bgroup(bt, 2)
                    prev = bt
                else:
                    # drain the final batch's outputs as they appear
                    bgroup(bt, 2, nc.sync, [4, 5])
                    bgroup(bt, 3, nc.scalar, [6, 7])

    nc.compile()
    return nc


def _numpy_fallback(x):
    """Reference math on host — only for unexpected shapes/degenerate L."""
    B, S, D = x.shape
    C = _dct_matrix64(S).astype(np.float32)
    x_dct = np.tensordot(C, x, axes=([1], [1])).transpose(1, 0, 2)  # (B,S,D)
    m = np.abs(x_dct.mean(axis=0).mean(axis=1))
    thr = np.quantile(m, 0.7)
    idx = np.nonzero(m > thr)[0]
    last_index = int(idx[-1]) if idx.size > 0 else -1
    trunc = x_dct[:, :last_index, :]
    L = trunc.shape[1]
    Cl = _dct_matrix64(L).astype(np.float32)
    recon = np.tensordot(Cl.T, trunc, axes=([1], [1])).transpose(1, 0, 2)
    return recon.astype(np.float16), np.ascontiguousarray(trunc)


def kernel(x, _trace=False):
    global LAST_RESULTS
    x = np.ascontiguousarray(np.asarray(x), dtype=np.float32)
    if x.shape != (_B, _S, _D):
        return _numpy_fallback(x)

    L = _resolve_L(x)
    FE = (L + 1) // 2
    # device path assumes 3 j-chunks per parity and FE <= 288
    if L < 8 or L >= _S or not (2 * _JT < FE <= 3 * _JT):
        return _numpy_fallback(x)

    Bc = _B // _NCORES
    key = (Bc, _S, _D, L)
    if key not in _CACHE:
        _CACHE[key] = _build_program(Bc, _S, _D, L)
    nc = _CACHE[key]

    _ensure_paths()
    if not _trace:
        os.environ["BASS_NEVER_TRACE"] = "1"
    from concourse.bass_utils import run_bass_kernel_spmd

    H = _S // 2
    FO = L // 2
    W = _build_weights(_S, L)
    xf = x[:, :H, :]
    xb = x[:, _S - 1:H - 1:-1, :]
    u = (xf + xb).astype(np.float16)
    v = (xf - xb).astype(np.float16)
    uv = np.empty((_B, 2, 3 * _KM, _D), dtype=np.float16)
    for pi, arr in ((0, u), (1, v)):
        uv[:, pi, :2 * _KM] = arr[:, :2 * _KM]
        for r in range(4):
            uv[:, pi, 2 * _KM + r * _KR:2 * _KM + (r + 1) * _KR] = \
                arr[:, 2 * _KM:]
    in_maps = []
    for i in range(_NCORES):
        m = {"uv": uv[i * Bc:(i + 1) * Bc]}
        m.update(W)
        in_maps.append(m)
    res = run_bass_kernel_spmd(nc, in_maps, list(range(_NCORES)), trace=_trace)
    LAST_RESULTS = res

    trunc = np.empty((_B, L, _D), dtype=np.float32)
    recon = np.empty((_B, L, _D), dtype=np.float16)
    for i in range(_NCORES):
        sl = slice(i * Bc, (i + 1) * Bc)
        tr = res.results[i]["tr"]          # [Bc, 2, H, D] f16
        trunc[sl, 0::2] = tr[:, 0, :FE]
        trunc[sl, 1::2] = tr[:, 1, :FO]
        gh = res.results[i]["gh"]          # [Bc, D, 2*NP] f16
        g = gh[:, :, :FE].transpose(0, 2, 1)
        h = gh[:, :, H:H + FE].transpose(0, 2, 1)
        recon[sl, :FE] = g + h
        recon[sl, FE:] = (g - h)[:, L - 1 - FE::-1]
    return recon, trunc
